# revision 9
# baseline (speedup 1.0000x reference)
"""Trainium2 Bass kernel for nn_CNN_88287347736632 (dense_cnn).

kernel(**inputs) takes the FULL unsharded inputs (as produced by
reference.setup_inputs) and returns the FULL [16, 108, 25] float32 output.

Sharding: pure data parallel over 8 NeuronCores — batch rows 2k, 2k+1 go to
core k. All conv/BN parameters are replicated (BN is folded into conv
weights/bias on the host).

Per-core mapping:
  - 216 sliding windows ([1,144,15] images), processed in waves of 16
    (one partial 12-window wave per batch row: 108 = 6*16 + 12).
  - Convs are tensor-engine matmuls; taps (dh,dw) are accumulating PSUM
    passes reading AP-shifted views of padded SBUF activations;
    tile_position packing runs up to 16 32x32 PE tiles (16 windows)
    concurrently.
  - Weight loads: one full-array 128-column LDWEIGHTS per (chunk, tap)
    group loads all 16 tiles at once (the tiled weight tensors replicate
    the 32x32 block across the 4x4 grid); the per-matmul narrow LDWEIGHTS
    that bass emits are removed by a post-legalization dedupe pass that
    models the PE array weight state and only drops provably-redundant
    loads.
  - PSUM evacuation (bias+ReLU+bf16 downcast) is split between the ACT
    engine (activation) and the DVE (tensor_scalar add/max) so neither is
    a serial bottleneck; the (2,1) max-pool after conv4 runs on GPSIMD.
  - conv7 (12x9 kernel, K=64*12*9) uses a 2x h-replicated layout giving 54
    full-K=128 passes at M=128; flushes are PAIRED (two 7-window groups
    interleaved per tap index) so each w7 column load is reused twice.
  - All matmul operands bf16 (fp32 PSUM accumulation); end-to-end rel err
    vs the fp32 reference is ~1.7e-3.
  - mean-pool and the 1x1 conv8 commute: the ACT-engine evacuation of conv7
    output computes spatial sums via accum_out, and conv8 is a single K=128
    matmul over all 216 window-sums.
"""

import numpy as np
import ml_dtypes

import concourse.bass as bass
import concourse.mybir as mybir
import concourse.tile as tile
from concourse import bacc
from concourse.bass_utils import run_bass_kernel_spmd

BF16 = mybir.dt.bfloat16
F32 = mybir.dt.float32
RELU = mybir.ActivationFunctionType.Relu
IDENT = mybir.ActivationFunctionType.Identity
ADD = mybir.AluOpType.add
MAX = mybir.AluOpType.max

EPS = 1e-5
CTX = 7
F = 144
WIN = 15
T_PER_B = 108
N_CORES = 8

DEDUPE_LDW = True

CHUNKS_144 = [(0, 29), (29, 29), (58, 29), (87, 29), (116, 28)]
CHUNKS_70 = [(0, 35), (35, 35)]
CHUNKS_68 = [(0, 34), (34, 34)]


# ----------------------------------------------------------------- host prep

def fold_bn(inputs):
    Ws, bs = {}, {}
    for i in range(1, 8):
        W = np.asarray(inputs[f'W{i}'], np.float32)
        b = np.asarray(inputs[f'b{i}'], np.float32)
        g = np.asarray(inputs[f'g{i}'], np.float32)
        be = np.asarray(inputs[f'be{i}'], np.float32)
        m = np.asarray(inputs[f'm{i}'], np.float32)
        v = np.asarray(inputs[f'v{i}'], np.float32)
        s = g / np.sqrt(v + EPS)
        Ws[i] = W * s[:, None, None, None]
        bs[i] = (b - m) * s + be
    return Ws, bs


def wave_plan(T=T_PER_B):
    plan = []
    t0 = 0
    while T - t0 > 12:
        plan.append((t0, 4))
        t0 += 16
    assert T - t0 in (12, 8, 4)
    plan.append((t0, (T - t0) // 4))
    return plan


def build_x9(xb, plan):
    """Host im2col for conv1, one batch row. Layout per wave:
    [g(4)][k(9)][slot(ncols)][f(144)][j(15)]; window w = slot*4+g at t0+w."""
    T = xb.shape[0]
    xpad = np.pad(xb, ((CTX, CTX), (0, 0)))
    WINDOWS = np.lib.stride_tricks.sliding_window_view(xpad, 15, axis=0)
    WP = np.zeros((T, F + 2, WIN + 2), np.float32)
    WP[:, 1:F + 1, 1:WIN + 1] = WINDOWS
    out = []
    for (t0, ncols) in plan:
        for g in range(4):
            for dh in range(3):
                for dw in range(3):
                    for slot in range(ncols):
                        t = t0 + slot * 4 + g
                        out.append(WP[t, dh:dh + F, dw:dw + WIN].ravel())
    return np.concatenate(out).astype(ml_dtypes.bfloat16)


def prep_weights(Ws, bs, W8, b8):
    d = {}
    # l1: tiled [128,128]: block (32g:32g+9, 32Ti:32Ti+32) = 9 taps x 32 outch
    w1blk = np.zeros((32, 32), np.float32)
    for dh in range(3):
        for dw in range(3):
            w1blk[dh * 3 + dw, :] = Ws[1][:, 0, dh, dw]
    d['w1t'] = np.tile(w1blk, (4, 4)).astype(ml_dtypes.bfloat16)
    # l2-4: per tap k a [128,128] full-array tile = 4x4 replication of Wk.T
    for l in (2, 3, 4):
        w = np.zeros((128, 9 * 128), np.float32)
        for k in range(9):
            dh, dw = k // 3, k % 3
            w[:, 128 * k:128 * k + 128] = np.tile(Ws[l][:, :, dh, dw].T, (4, 4))
        d[f'w{l}t'] = w.astype(ml_dtypes.bfloat16)
    # l5: per tap 4x2 replication of (32 in x 64 out)
    w5 = np.zeros((128, 9 * 128), np.float32)
    for k in range(9):
        dh, dw = k // 3, k % 3
        w5[:, 128 * k:128 * k + 128] = np.tile(Ws[5][:, :, dh, dw].T, (4, 2))
    d['w5t'] = w5.astype(ml_dtypes.bfloat16)
    # l6: per tap 2x2 replication of (64 in x 64 out)
    w6 = np.zeros((128, 9 * 128), np.float32)
    for k in range(9):
        dh, dw = k // 3, k % 3
        w6[:, 128 * k:128 * k + 128] = np.tile(Ws[6][:, :, dh, dw].T, (2, 2))
    d['w6t'] = w6.astype(ml_dtypes.bfloat16)
    w7 = np.zeros((128, 54 * 128), np.float32)
    for jj in range(6):
        for dw in range(9):
            idx = jj * 9 + dw
            for p in range(2):
                w7[64 * p:64 * p + 64, 128 * idx:128 * idx + 128] = \
                    Ws[7][:, :, 2 * jj + p, dw].T
    d['w7'] = w7.astype(ml_dtypes.bfloat16)
    d['w8'] = (np.asarray(W8, np.float32)[:, :, 0, 0].T / 69.0).astype(np.float32)
    for l, c in ((1, 32), (2, 32), (3, 32), (4, 32), (5, 64), (6, 64)):
        t = np.zeros((128, 1), np.float32)
        t[:, 0] = np.tile(bs[l], 128 // c)
        d[f'b{l}'] = t
    d['b7'] = bs[7].reshape(128, 1).astype(np.float32)
    d['b8'] = np.asarray(b8, np.float32).reshape(25, 1)
    return d


def host_prepare(inputs, n_cores=N_CORES):
    Ws, bs = fold_bn(inputs)
    wd = prep_weights(Ws, bs, inputs['W8'], inputs['b8'])
    x = np.asarray(inputs['x'], np.float32)
    B = x.shape[0]
    b_per_core = B // n_cores
    plan = wave_plan(x.shape[1])
    in_maps = []
    for c in range(n_cores):
        x9s = [build_x9(x[c * b_per_core + i], plan) for i in range(b_per_core)]
        m = dict(wd)
        m['x9'] = np.concatenate(x9s)
        in_maps.append(m)
    return in_maps, plan


# ----------------------------------------------------- ldweights dedupe pass

def dedupe_ldweights(nc):
    """Remove InstLdweights whose content is provably already resident in the
    PE array. Walks each block in final (post-legalization) order and models
    per-32x32-tile weight state; only drops loads whose every covered tile
    already holds identical content (same memref/partition strip/column
    offset, resident rows >= new rows). Conservative: any unknown PE-array
    mutation resets state."""
    removed = 0
    # names referenced as dependencies anywhere must not be removed
    refset = set()
    for fn in nc.m.functions:
        for b in fn.blocks:
            for i in b.instructions:
                try:
                    refset.update(i.sync_dependency_names())
                    refset.update(i.nosync_dependency_names())
                except Exception:
                    pass
    for fn in nc.m.functions:
        for b in fn.blocks:
            resident = {}
            drop = set()
            for i in b.instructions:
                tn = type(i).__name__
                if tn == 'InstMatmult':
                    if getattr(i, 'is_transpose', None):
                        resident = {}
                    continue
                if tn != 'InstLdweights':
                    continue
                ok = True
                sigs = None
                try:
                    if getattr(i, 'perf_mode', None) is not None or \
                       getattr(i, 'is_transpose', None):
                        ok = False
                    ap = i.ins[0]
                    bap = ap.bass_ap
                    memref = ap.memref
                    part0 = bap.base_partition()
                    off = bap.offset
                    nrows = bap.partition_size()
                    ncols = bap.free_size()
                    tp = i.tile_position or (part0, 0)
                    R, C = tp
                    if ok and (R % 32 == 0 and C % 32 == 0 and part0 == R
                               and ncols % 32 == 0):
                        # contiguous column check: innermost stride must be 1
                        aplist = [list(p) for p in bap.ap]
                        if aplist[-1][0] != 1 or aplist[0][1] != nrows:
                            ok = False
                    else:
                        ok = False
                    if ok:
                        # bap.offset is a flat element offset that includes
                        # the partition component; subtract it to get the
                        # column offset within a partition.
                        col0 = off - part0 * aplist[0][0]
                        sigs = {}
                        nstrip = (nrows + 31) // 32
                        nct = ncols // 32
                        for sr in range(nstrip):
                            rrows = min(32, nrows - 32 * sr)
                            for sc in range(nct):
                                tile_key = (R // 32 + sr, C // 32 + sc)
                                sigs[tile_key] = (memref, R + 32 * sr,
                                                  col0 + 32 * sc, rrows)
                except Exception:
                    ok = False
                if not ok or sigs is None:
                    resident = {}
                    continue
                if i.name not in refset and all(
                        k in resident
                        and resident[k][:3] == s[:3]
                        and resident[k][3] >= s[3]
                        for k, s in sigs.items()):
                    drop.add(id(i))
                    removed += 1
                else:
                    resident.update(sigs)
            if drop:
                b.instructions = [x for x in b.instructions
                                  if id(x) not in drop]
    return removed


# ------------------------------------------------------------- device builder

class ActTile:
    def __init__(self, ap, gsize, windows):
        self.ap = ap
        self.gsize = gsize
        self.windows = windows


def emit(tc, ins, y_ap, n_b=2, T=T_PER_B, repeat=1, stop_after=None):
    nc = tc.nc
    _ctr = [0]

    def nm(base):
        _ctr[0] += 1
        return f"{base}{_ctr[0]}"
    plan = wave_plan(T)
    nwin_total = n_b * T

    import contextlib
    stack = contextlib.ExitStack()
    persist = stack.enter_context(tc.tile_pool(name="persist", bufs=1))
    x9pool = stack.enter_context(tc.tile_pool(name="x9", bufs=2))
    a5p_pool = stack.enter_context(tc.tile_pool(name="a5p", bufs=6))
    a5i_pool = stack.enter_context(tc.tile_pool(name="a5i", bufs=6))
    a6_pool = stack.enter_context(tc.tile_pool(name="a6", bufs=6))
    a7_pool = stack.enter_context(tc.tile_pool(name="a7", bufs=8))
    rep_pool = stack.enter_context(tc.tile_pool(name="rep7", bufs=4))
    psA = stack.enter_context(tc.tile_pool(name="psA", bufs=6, space="PSUM"))
    psB = stack.enter_context(tc.tile_pool(name="psB", bufs=2, space="PSUM"))

    wt = {}
    for name, shape, dt in (
        ('w1t', [128, 128], BF16), ('w2t', [128, 9 * 128], BF16),
        ('w3t', [128, 9 * 128], BF16), ('w4t', [128, 9 * 128], BF16),
        ('w5t', [128, 9 * 128], BF16), ('w6t', [128, 9 * 128], BF16),
        ('w7', [128, 54 * 128], BF16), ('w8', [128, 25], F32),
        ('b1', [128, 1], F32), ('b2', [128, 1], F32), ('b3', [128, 1], F32),
        ('b4', [128, 1], F32), ('b5', [128, 1], F32), ('b6', [128, 1], F32),
        ('b7', [128, 1], F32), ('b8', [25, 1], F32),
    ):
        t = persist.tile(shape, dt, tag=name, name=nm(name))
        nc.sync.dma_start(t[:], ins[name])
        wt[name] = t

    PADF = 146 * 17
    pad_tiles = {}
    for l in (2, 3, 4):
        for i in range(4):
            t = persist.tile([128, PADF], BF16, tag=f"act{l}_{i}", name=nm("pad"))
            v = t[:].rearrange("p (h w) -> p h w", h=146, w=17)
            nc.vector.memset(v[:, 0, :], 0.0)
            nc.vector.memset(v[:, 145, :], 0.0)
            nc.vector.memset(v[:, :, 0], 0.0)
            nc.vector.memset(v[:, :, 16], 0.0)
            pad_tiles[(l, i)] = t

    act9 = persist.tile([128, nwin_total], F32, tag="act9", name="act9")
    dummy8 = persist.tile([128, 8 * 69], F32, tag="dummy8", name="dummy8")
    out_sb = persist.tile([25, nwin_total], F32, tag="out_sb", name="out_sb")

    win_order = []

    def evac(eng_idx, dst, src, bias_ap):
        """bias + ReLU + downcast evacuation on alternating engines."""
        if eng_idx == 0:
            nc.scalar.activation(dst, src, RELU, bias=bias_ap)
        else:
            nc.vector.tensor_scalar(out=dst, in0=src, scalar1=bias_ap,
                                    scalar2=0.0, op0=ADD, op1=MAX)

    def conv_pad_layer(l, in_tiles, out_is_pad, bias, w_t):
        n_t = len(in_tiles)
        n_g = max(len(t.windows) for t in in_tiles)
        outs = []
        for g in range(n_g):
            if out_is_pad:
                ot = pad_tiles[(l + 1, g)]
                outs.append(ActTile(ot[:], 32, {}))
            else:
                ot = a5p_pool.tile([128, 144 * 15], BF16, tag="a5p", name=nm("a5p"))
                outs.append(ActTile(ot[:], 32, {}))
        wv = w_t[:].rearrange("p (k c) -> p k c", k=9, c=128)
        for (h0, hc) in CHUNKS_144:
            N = hc * 15
            pss = [psA.tile([128, 512], F32, tag="mm", name=nm("mm")) for _ in range(n_g)]
            for k in range(9):
                dh, dw = k // 3, k % 3
                # dummy 1-col matmul: its auto-LDWEIGHTS is a single FWL
                # 128-col load covering all 16 tiles; anchored to this
                # group's PSUM bank so the scheduler can't hoist it.
                nc.tensor.matmul(pss[0][:, 508:509], wv[:, k, :],
                                 wv[:, k, 0:1], start=False, stop=False,
                                 skip_group_check=True, tile_position=(0, 0))
                for g in range(n_g):
                    for Ti, it in enumerate(in_tiles):
                        if g not in it.windows:
                            continue
                        iv = it.ap.rearrange("p (h w) -> p h w", h=146, w=17)
                        rhs = iv[32 * g:32 * g + 32, h0 + dh:h0 + dh + hc, dw:dw + 15]
                        nc.tensor.matmul(
                            pss[g][32 * Ti:32 * Ti + 32, 0:N],
                            wv[32 * g:32 * g + 32, k, 32 * Ti:32 * Ti + 32],
                            rhs, start=(k == 0), stop=(k == 8), skip_group_check=True,
                            tile_position=(32 * g, 32 * Ti))
            for g in range(n_g):
                np_used = 32 * n_t
                src = pss[g][0:np_used, 0:N].rearrange("p (h w) -> p h w", h=hc, w=15)
                if out_is_pad:
                    ov = outs[g].ap.rearrange("p (h w) -> p h w", h=146, w=17)
                    dst = ov[0:np_used, 1 + h0:1 + h0 + hc, 1:16]
                else:
                    ov = outs[g].ap.rearrange("p (h w) -> p h w", h=144, w=15)
                    dst = ov[0:np_used, h0:h0 + hc, :]
                evac(g % 2, dst, src, bias[0:np_used, :])
        for g in range(n_g):
            for Ti, it in enumerate(in_tiles):
                if g in it.windows:
                    outs[g].windows[Ti] = it.windows[g]
        return outs

    def l1_wave(x9t, t0, ncols, b_idx):
        xv = x9t[:].rearrange("p (c h w) -> p c h w", c=4, h=F, w=WIN)
        outs = [ActTile(pad_tiles[(2, g)][:], 32, {}) for g in range(4)]
        for (h0, hc) in CHUNKS_144:
            N = hc * 15
            pss = [psA.tile([128, 512], F32, tag="mm", name=nm("mm")) for _ in range(4)]
            nc.tensor.matmul(pss[0][:, 508:509], wt['w1t'][:],
                             wt['w1t'][:, 0:1], start=False, stop=False,
                             skip_group_check=True, tile_position=(0, 0))
            for g in range(4):
                for T_ in range(ncols):
                    nc.tensor.matmul(
                        pss[g][32 * T_:32 * T_ + 32, 0:N],
                        wt['w1t'][32 * g:32 * g + 9, 32 * T_:32 * T_ + 32],
                        xv[32 * g:32 * g + 9, T_, h0:h0 + hc, :],
                        start=True, stop=True, skip_group_check=True,
                        tile_position=(32 * g, 32 * T_))
            for g in range(4):
                np_used = 32 * ncols
                src = pss[g][0:np_used, 0:N].rearrange("p (h w) -> p h w", h=hc, w=15)
                ov = outs[g].ap.rearrange("p (h w) -> p h w", h=146, w=17)
                evac(g % 2, ov[0:np_used, 1 + h0:1 + h0 + hc, 1:16], src,
                     wt['b1'][0:np_used, :])
        for g in range(4):
            for T_ in range(ncols):
                outs[g].windows[T_] = b_idx * T + t0 + T_ * 4 + g
        return outs

    def pool1(a5p_tiles):
        outs = []
        for t in a5p_tiles:
            np_used = 32 * (max(t.windows) + 1)
            o = a5i_pool.tile([128, 72 * 15], BF16, tag="a5i", name=nm("a5i"))[:]
            sv = t.ap.rearrange("p (h two w) -> p h two w", h=72, two=2, w=15)
            ov = o.rearrange("p (h w) -> p h w", h=72, w=15)
            nc.vector.tensor_max(ov[0:np_used], sv[0:np_used, :, 0, :],
                                 sv[0:np_used, :, 1, :])
            outs.append(ActTile(o, 32, dict(t.windows)))
        return outs

    def l5_wave(tiles):
        pair = len(tiles) == 2
        a6_tiles = []
        n_g = len(tiles[0].windows)
        for g in range(n_g):
            o = a6_pool.tile([128, 70 * 13], BF16, tag="a6", name=nm("a6"))[:]
            a6_tiles.append(ActTile(o, 64, {}))
        wv = wt['w5t'][:].rearrange("p (k c) -> p k c", k=9, c=128)
        for (h0, hc) in CHUNKS_70:
            N = hc * 13
            pss = [psA.tile([128, 512], F32, tag="mm", name=nm("mm")) for _ in range(n_g)]
            for k in range(9):
                dh, dw = k // 3, k % 3
                nc.tensor.matmul(pss[0][:, 508:509], wv[:, k, :],
                                 wv[:, k, 0:1], start=False, stop=False,
                                 skip_group_check=True, tile_position=(0, 0))
                for g in range(n_g):
                    for c5, it in enumerate(tiles):
                        iv = it.ap.rearrange("p (h w) -> p h w", h=72, w=15)
                        rhs = iv[32 * g:32 * g + 32, h0 + dh:h0 + dh + hc, dw:dw + 13]
                        nc.tensor.matmul(
                            pss[g][64 * c5:64 * c5 + 64, 0:N],
                            wv[32 * g:32 * g + 32, k, 64 * c5:64 * c5 + 64],
                            rhs, start=(k == 0), stop=(k == 8), skip_group_check=True,
                            tile_position=(32 * g, 64 * c5))
            for g in range(n_g):
                np_used = 128 if pair else 64
                src = pss[g][0:np_used, 0:N].rearrange("p (h w) -> p h w", h=hc, w=13)
                ov = a6_tiles[g].ap.rearrange("p (h w) -> p h w", h=70, w=13)
                if pair:
                    dst = ov[:, h0:h0 + hc, :]
                else:
                    dst = ov[0:64, h0:h0 + hc, :]
                nc.scalar.activation(dst, src, RELU, bias=wt['b5'][0:np_used, :])
        for g in range(n_g):
            for c5, it in enumerate(tiles):
                a6_tiles[g].windows[c5] = it.windows[g]
        if not pair:
            full = []
            for j in range(0, n_g, 2):
                t0_, t1_ = a6_tiles[j], a6_tiles[j + 1]
                o = a6_pool.tile([128, 70 * 13], BF16, tag="a6", name=nm("a6"))[:]
                nc.vector.tensor_copy(o[0:64, :], t0_.ap[0:64, :])
                nc.vector.tensor_copy(o[64:128, :], t1_.ap[0:64, :])
                full.append(ActTile(o, 64, {0: t0_.windows[0], 1: t1_.windows[0]}))
            a6_tiles = full
        return a6_tiles

    def l6_wave(tP, tQ):
        a7_tiles = [ActTile(a7_pool.tile([128, 68 * 11], BF16, tag="a7",
                                         name=nm("a7"))[:], 64, {})
                    for _ in range(2)]
        wv = wt['w6t'][:].rearrange("p (k c) -> p k c", k=9, c=128)
        for (h0, hc) in CHUNKS_68:
            N = hc * 11
            pss = [psA.tile([128, 512], F32, tag="mm", name=nm("mm")) for _ in range(2)]
            for k in range(9):
                dh, dw = k // 3, k % 3
                nc.tensor.matmul(pss[0][:, 508:509], wv[:, k, :],
                                 wv[:, k, 0:1], start=False, stop=False,
                                 skip_group_check=True, tile_position=(0, 0))
                for c in range(2):
                    for cc, it in enumerate((tP, tQ)):
                        iv = it.ap.rearrange("p (h w) -> p h w", h=70, w=13)
                        rhs = iv[64 * c:64 * c + 64, h0 + dh:h0 + dh + hc, dw:dw + 11]
                        nc.tensor.matmul(
                            pss[c][64 * cc:64 * cc + 64, 0:N],
                            wv[64 * c:64 * c + 64, k, 64 * cc:64 * cc + 64],
                            rhs, start=(k == 0), stop=(k == 8), skip_group_check=True,
                            tile_position=(64 * c, 64 * cc))
            for c in range(2):
                src = pss[c][:, 0:N].rearrange("p (h w) -> p h w", h=hc, w=11)
                ov = a7_tiles[c].ap.rearrange("p (h w) -> p h w", h=68, w=11)
                nc.scalar.activation(ov[:, h0:h0 + hc, :], src, RELU,
                                     bias=wt['b6'][:])
        for c in range(2):
            a7_tiles[c].windows = {0: tP.windows[c], 1: tQ.windows[c]}
        return a7_tiles

    REP_S = 7
    rep_state = {"tiles": []}

    def rep_flush():
        tiles = rep_state["tiles"]
        rep_state["tiles"] = []
        if not tiles:
            return
        w7v = wt['w7'][:].rearrange("p (i c) -> p i c", i=54, c=128)
        pss = []
        for t in tiles:
            S = len(t["slots"])
            ps7 = psB.tile([128, REP_S * 69], F32, tag="l7", name=nm("ps7"))[:]
            pv = ps7.rearrange("p (s h w) -> p s h w", s=REP_S, h=23, w=3)
            pss.append((t, S, pv))
        for jj in range(6):
            for dw in range(9):
                idx = jj * 9 + dw
                for (t, S, pv) in pss:
                    nc.tensor.matmul(
                        pv[:, 0:S, :, :], w7v[:, idx, :],
                        t["rv"][:, 0:S, 2 * jj:2 * jj + 23, dw:dw + 3],
                        start=(idx == 0), stop=(idx == 53), skip_group_check=True,
                        tile_position=(0, 0))
        dv = dummy8[:].rearrange("p (s n) -> p s n", s=8, n=69)
        for (t, S, pv) in pss:
            for s in range(S):
                col = len(win_order)
                win_order.append(t["slots"][s])
                nc.scalar.activation(
                    dv[:, col % 8, :], pv[:, s, :, :].rearrange("p h w -> p (h w)"),
                    RELU, bias=wt['b7'][:],
                    accum_out=act9[:, col:col + 1])

    def l7_push(tv, c, win):
        """Append one window's pooled conv6 output into the rep tiles."""
        tiles = rep_state["tiles"]
        if not tiles or len(tiles[-1]["slots"]) == REP_S:
            rep = rep_pool.tile([128, REP_S * 33 * 11], BF16, tag="rep7",
                                name=nm("rep7"))[:]
            rv = rep.rearrange("p (s h w) -> p s h w", s=REP_S, h=33, w=11)
            tiles.append({"rv": rv, "slots": []})
        cur = tiles[-1]
        s = len(cur["slots"])
        cur["slots"].append(win)
        for p in range(2):
            nc.vector.tensor_max(
                cur["rv"][64 * p:64 * p + 64, s, :, :],
                tv[64 * c:64 * c + 64, p:p + 33, 0, :],
                tv[64 * c:64 * c + 64, p:p + 33, 1, :])
        if len(tiles) == 2 and len(tiles[-1]["slots"]) == REP_S:
            rep_flush()

    def l7_group(a7_pair):
        for t in a7_pair:
            tv = t.ap.rearrange("p (h two w) -> p h two w", h=34, two=2, w=11)
            for c in range(2):
                l7_push(tv, c, t.windows[c])

    _dbg_row = [0]
    for rep in range(repeat):
        if rep > 0:
            win_order.clear()
        x9_off = 0
        a5i_queue = []
        a6_queue = []

        def drain_a6():
            while len(a6_queue) >= 2:
                tP = a6_queue.pop(0)
                tQ = a6_queue.pop(0)
                a7s = l6_wave(tP, tQ)
                if stop_after == 'l6':
                    for tl in a7s:
                        r = _dbg_row[0] % nwin_total
                        _dbg_row[0] += 1
                        nc.gpsimd.dma_start(
                            y_ap[r:r + 1, 0:25].rearrange("a c -> c a"),
                            tl.ap[0:25, 0:1])
                    continue
                l7_group(a7s)

        def drain_a5(force=False):
            while len(a5i_queue) >= 2 or (force and a5i_queue):
                tiles = [a5i_queue.pop(0)]
                if a5i_queue:
                    tiles.append(a5i_queue.pop(0))
                a6_queue.extend(l5_wave(tiles))
                drain_a6()

        for b_idx in range(n_b):
            for wi, (t0, ncols) in enumerate(plan):
                nelem = 4 * 9 * ncols * F * WIN
                x9t = x9pool.tile([128, 4 * F * WIN], BF16, tag="x9", name=nm("x9"))
                per_g = nelem // 4
                for g in range(4):
                    src = ins['x9'][x9_off + g * per_g: x9_off + (g + 1) * per_g]
                    dst = x9t[:].rearrange("p (c h w) -> p c h w", c=4, h=F, w=WIN)
                    nc.sync.dma_start(
                        dst[32 * g:32 * g + 9, 0:ncols, :, :],
                        src.rearrange("(k c h w) -> k c h w", k=9, c=ncols, h=F, w=WIN))
                x9_off += nelem
                tiles = l1_wave(x9t, t0, ncols, b_idx)
                tiles = conv_pad_layer(2, tiles, True, wt['b2'], wt['w2t'])
                tiles = conv_pad_layer(3, tiles, True, wt['b3'], wt['w3t'])
                tiles = conv_pad_layer(4, tiles, False, wt['b4'], wt['w4t'])
                a5i_queue.extend(pool1(tiles))
                if stop_after == 'l4':
                    while a5i_queue:
                        tl = a5i_queue.pop(0)
                        r = _dbg_row[0] % nwin_total
                        _dbg_row[0] += 1
                        nc.gpsimd.dma_start(
                            y_ap[r:r + 1, 0:25].rearrange("a c -> c a"),
                            tl.ap[0:25, 0:1])
                    continue
                drain_a5()
            drain_a5(force=True)
            drain_a6()
        if stop_after is not None:
            a5i_queue.clear()
            a6_queue.clear()
            continue
        rep_flush()
        assert not a5i_queue and not a6_queue
        assert len(win_order) == nwin_total

        ps8 = psB.tile([25, nwin_total], F32, tag="l7", name=nm("ps8"))
        nc.tensor.matmul(ps8[:, :], wt['w8'][:, 0:25], act9[:, :],
                         start=True, stop=True)
        nc.scalar.activation(out_sb[:, :], ps8[:, :], IDENT, bias=wt['b8'][:])
        nc.sync.dma_start(y_ap.rearrange("t c -> c t"), out_sb[:, :])

    stack.close()
    return win_order


# --------------------------------------------------------------- entry point

_CACHE = {}


def build_program(in_map, n_b=2, T=T_PER_B, repeat=1, stop_after=None):
    """Build + bacc-compile the SPMD program. Returns (nc, win_order)."""
    nc = bacc.Bacc("TRN2", target_bir_lowering=False, debug=False,
                   num_devices=N_CORES)
    dram = {}
    for name, arr in in_map.items():
        dram[name] = nc.dram_tensor(name, list(arr.shape),
                                    mybir.dt.from_np(arr.dtype),
                                    kind="ExternalInput")
    nwin = n_b * T
    y = nc.dram_tensor("y", [nwin, 25], mybir.dt.float32, kind="ExternalOutput")
    with tile.TileContext(nc) as tc:
        win_order = emit(tc, {k: v.ap() for k, v in dram.items()}, y.ap(),
                         n_b=n_b, T=T, repeat=repeat, stop_after=stop_after)
    if DEDUPE_LDW:
        dedupe_ldweights(nc)
    nc.compile()
    return nc, win_order


def kernel(**inputs):
    x = np.asarray(inputs['x'])
    B, T, _ = x.shape
    in_maps, _plan = host_prepare(inputs, n_cores=N_CORES)
    key = (B, T)
    if key not in _CACHE:
        _CACHE[key] = build_program(in_maps[0], n_b=B // N_CORES, T=T)
    nc, win_order = _CACHE[key]
    res = run_bass_kernel_spmd(nc, in_maps, list(range(N_CORES)))
    order = np.asarray(win_order)
    b_per_core = B // N_CORES
    out = np.zeros((B, T, 25), np.float32)
    for c in range(N_CORES):
        yc = np.zeros((b_per_core * T, 25), np.float32)
        yc[order] = res.results[c]['y']
        out[c * b_per_core:(c + 1) * b_per_core] = yc.reshape(b_per_core, T, 25)
    return out


# revision 10
# speedup vs baseline: 1.1524x; 1.1524x over previous
"""Trainium2 Bass kernel for nn_CNN_88287347736632 (dense_cnn).

kernel(**inputs) takes the FULL unsharded inputs (as produced by
reference.setup_inputs) and returns the FULL [16, 108, 25] float32 output.

Sharding: pure data parallel over 8 NeuronCores — batch rows 2k, 2k+1 go to
core k. All conv/BN parameters are replicated (BN is folded into conv
weights/bias on the host).

Per-core mapping:
  - 216 sliding windows ([1,144,15] images), processed in waves of 16
    (one partial 12-window wave per batch row: 108 = 6*16 + 12).
  - Convs are tensor-engine matmuls; taps (dh,dw) are accumulating PSUM
    passes reading AP-shifted views of padded SBUF activations;
    tile_position packing runs up to 16 32x32 PE tiles (16 windows)
    concurrently.
  - Weight loads: one full-array 128-column LDWEIGHTS per (chunk, tap)
    group loads all 16 tiles at once (the tiled weight tensors replicate
    the 32x32 block across the 4x4 grid); the per-matmul narrow LDWEIGHTS
    that bass emits are removed by a post-legalization dedupe pass that
    models the PE array weight state and only drops provably-redundant
    loads.
  - PSUM evacuation (bias+ReLU+bf16 downcast) is split between the ACT
    engine (activation) and the DVE (tensor_scalar add/max) so neither is
    a serial bottleneck.
  - conv7 (12x9 kernel, K=64*12*9) uses a 2x h-replicated layout giving 54
    full-K=128 passes at M=128; flushes are PAIRED (two 7-window groups
    interleaved per tap index) so each w7 column load is reused twice.
  - All matmul operands bf16 (fp32 PSUM accumulation); end-to-end rel err
    vs the fp32 reference is ~1.7e-3.
  - mean-pool and the 1x1 conv8 commute: the ACT-engine evacuation of conv7
    output computes spatial sums via accum_out, and conv8 is a single K=128
    matmul over all 216 window-sums.
"""

import numpy as np
import ml_dtypes

import concourse.bass as bass
import concourse.mybir as mybir
import concourse.tile as tile
from concourse import bacc
from concourse.bass_utils import run_bass_kernel_spmd

BF16 = mybir.dt.bfloat16
F32 = mybir.dt.float32
RELU = mybir.ActivationFunctionType.Relu
IDENT = mybir.ActivationFunctionType.Identity
ADD = mybir.AluOpType.add
MAX = mybir.AluOpType.max

EPS = 1e-5
CTX = 7
F = 144
WIN = 15
T_PER_B = 108
N_CORES = 8

DEDUPE_LDW = True

CHUNKS_144 = [(0, 29), (29, 29), (58, 29), (87, 29), (116, 28)]
CHUNKS_70 = [(0, 35), (35, 35)]
CHUNKS_68 = [(0, 34), (34, 34)]


# ----------------------------------------------------------------- host prep

def fold_bn(inputs):
    Ws, bs = {}, {}
    for i in range(1, 8):
        W = np.asarray(inputs[f'W{i}'], np.float32)
        b = np.asarray(inputs[f'b{i}'], np.float32)
        g = np.asarray(inputs[f'g{i}'], np.float32)
        be = np.asarray(inputs[f'be{i}'], np.float32)
        m = np.asarray(inputs[f'm{i}'], np.float32)
        v = np.asarray(inputs[f'v{i}'], np.float32)
        s = g / np.sqrt(v + EPS)
        Ws[i] = W * s[:, None, None, None]
        bs[i] = (b - m) * s + be
    return Ws, bs


def wave_plan(T=T_PER_B):
    plan = []
    t0 = 0
    while T - t0 > 12:
        plan.append((t0, 4))
        t0 += 16
    assert T - t0 in (12, 8, 4)
    plan.append((t0, (T - t0) // 4))
    return plan


def build_x9(xb, plan):
    """Host im2col for conv1, one batch row. Layout per wave:
    [g(4)][k(9)][slot(ncols)][f(144)][j(15)]; window w = slot*4+g at t0+w."""
    T = xb.shape[0]
    xpad = np.pad(xb, ((CTX, CTX), (0, 0)))
    WINDOWS = np.lib.stride_tricks.sliding_window_view(xpad, 15, axis=0)
    WP = np.zeros((T, F + 2, WIN + 2), np.float32)
    WP[:, 1:F + 1, 1:WIN + 1] = WINDOWS
    out = []
    for (t0, ncols) in plan:
        for g in range(4):
            for dh in range(3):
                for dw in range(3):
                    for slot in range(ncols):
                        t = t0 + slot * 4 + g
                        out.append(WP[t, dh:dh + F, dw:dw + WIN].ravel())
    return np.concatenate(out).astype(ml_dtypes.bfloat16)


def prep_weights(Ws, bs, W8, b8):
    d = {}
    # l1: tiled [128,128]: block (32g:32g+9, 32Ti:32Ti+32) = 9 taps x 32 outch
    w1blk = np.zeros((32, 32), np.float32)
    for dh in range(3):
        for dw in range(3):
            w1blk[dh * 3 + dw, :] = Ws[1][:, 0, dh, dw]
    d['w1t'] = np.tile(w1blk, (4, 4)).astype(ml_dtypes.bfloat16)
    # l2-4: per tap k a [128,128] full-array tile = 4x4 replication of Wk.T
    for l in (2, 3, 4):
        w = np.zeros((128, 9 * 128), np.float32)
        for k in range(9):
            dh, dw = k // 3, k % 3
            w[:, 128 * k:128 * k + 128] = np.tile(Ws[l][:, :, dh, dw].T, (4, 4))
        d[f'w{l}t'] = w.astype(ml_dtypes.bfloat16)
    # l5: per tap 4x2 replication of (32 in x 64 out)
    w5 = np.zeros((128, 9 * 128), np.float32)
    for k in range(9):
        dh, dw = k // 3, k % 3
        w5[:, 128 * k:128 * k + 128] = np.tile(Ws[5][:, :, dh, dw].T, (4, 2))
    d['w5t'] = w5.astype(ml_dtypes.bfloat16)
    # l6: per tap 2x2 replication of (64 in x 64 out)
    w6 = np.zeros((128, 9 * 128), np.float32)
    for k in range(9):
        dh, dw = k // 3, k % 3
        w6[:, 128 * k:128 * k + 128] = np.tile(Ws[6][:, :, dh, dw].T, (2, 2))
    d['w6t'] = w6.astype(ml_dtypes.bfloat16)
    w7 = np.zeros((128, 54 * 128), np.float32)
    for jj in range(6):
        for dw in range(9):
            idx = jj * 9 + dw
            for p in range(2):
                w7[64 * p:64 * p + 64, 128 * idx:128 * idx + 128] = \
                    Ws[7][:, :, 2 * jj + p, dw].T
    d['w7'] = w7.astype(ml_dtypes.bfloat16)
    d['w8'] = (np.asarray(W8, np.float32)[:, :, 0, 0].T / 69.0).astype(np.float32)
    for l, c in ((1, 32), (2, 32), (3, 32), (4, 32), (5, 64), (6, 64)):
        t = np.zeros((128, 1), np.float32)
        t[:, 0] = np.tile(bs[l], 128 // c)
        d[f'b{l}'] = t
    d['b7'] = bs[7].reshape(128, 1).astype(np.float32)
    d['b8'] = np.asarray(b8, np.float32).reshape(25, 1)
    return d


def host_prepare(inputs, n_cores=N_CORES):
    Ws, bs = fold_bn(inputs)
    wd = prep_weights(Ws, bs, inputs['W8'], inputs['b8'])
    x = np.asarray(inputs['x'], np.float32)
    B = x.shape[0]
    b_per_core = B // n_cores
    plan = wave_plan(x.shape[1])
    in_maps = []
    for c in range(n_cores):
        x9s = [build_x9(x[c * b_per_core + i], plan) for i in range(b_per_core)]
        m = dict(wd)
        m['x9'] = np.concatenate(x9s)
        in_maps.append(m)
    return in_maps, plan


# ----------------------------------------------------- ldweights dedupe pass

def dedupe_ldweights(nc):
    """Remove InstLdweights whose content is provably already resident in the
    PE array. Walks each block in final (post-legalization) order and models
    per-32x32-tile weight state; only drops loads whose every covered tile
    already holds identical content (same memref/partition strip/column
    offset, resident rows >= new rows). Conservative: any unknown PE-array
    mutation resets state."""
    removed = 0
    # names referenced as dependencies anywhere must not be removed
    refset = set()
    for fn in nc.m.functions:
        for b in fn.blocks:
            for i in b.instructions:
                try:
                    refset.update(i.sync_dependency_names())
                    refset.update(i.nosync_dependency_names())
                except Exception:
                    pass
    for fn in nc.m.functions:
        for b in fn.blocks:
            resident = {}
            drop = set()
            for i in b.instructions:
                tn = type(i).__name__
                if tn == 'InstMatmult':
                    if getattr(i, 'is_transpose', None):
                        resident = {}
                    continue
                if tn != 'InstLdweights':
                    continue
                ok = True
                sigs = None
                try:
                    if getattr(i, 'perf_mode', None) is not None or \
                       getattr(i, 'is_transpose', None):
                        ok = False
                    ap = i.ins[0]
                    bap = ap.bass_ap
                    memref = ap.memref
                    part0 = bap.base_partition()
                    off = bap.offset
                    nrows = bap.partition_size()
                    ncols = bap.free_size()
                    tp = i.tile_position or (part0, 0)
                    R, C = tp
                    if ok and (R % 32 == 0 and C % 32 == 0 and part0 == R
                               and ncols % 32 == 0):
                        # contiguous column check: innermost stride must be 1
                        aplist = [list(p) for p in bap.ap]
                        if aplist[-1][0] != 1 or aplist[0][1] != nrows:
                            ok = False
                    else:
                        ok = False
                    if ok:
                        # bap.offset is a flat element offset that includes
                        # the partition component; subtract it to get the
                        # column offset within a partition.
                        col0 = off - part0 * aplist[0][0]
                        sigs = {}
                        nstrip = (nrows + 31) // 32
                        nct = ncols // 32
                        for sr in range(nstrip):
                            rrows = min(32, nrows - 32 * sr)
                            for sc in range(nct):
                                tile_key = (R // 32 + sr, C // 32 + sc)
                                sigs[tile_key] = (memref, R + 32 * sr,
                                                  col0 + 32 * sc, rrows)
                except Exception:
                    ok = False
                if not ok or sigs is None:
                    resident = {}
                    continue
                if i.name not in refset and all(
                        k in resident
                        and resident[k][:3] == s[:3]
                        and resident[k][3] >= s[3]
                        for k, s in sigs.items()):
                    drop.add(id(i))
                    removed += 1
                else:
                    resident.update(sigs)
            if drop:
                b.instructions = [x for x in b.instructions
                                  if id(x) not in drop]
    return removed


# ------------------------------------------------------------- device builder

class ActTile:
    def __init__(self, ap, gsize, windows):
        self.ap = ap
        self.gsize = gsize
        self.windows = windows


def emit(tc, ins, y_ap, n_b=2, T=T_PER_B, repeat=1, stop_after=None):
    nc = tc.nc
    _ctr = [0]

    def nm(base):
        _ctr[0] += 1
        return f"{base}{_ctr[0]}"
    plan = wave_plan(T)
    nwin_total = n_b * T

    import contextlib
    stack = contextlib.ExitStack()
    persist = stack.enter_context(tc.tile_pool(name="persist", bufs=1))
    x9pool = stack.enter_context(tc.tile_pool(name="x9", bufs=2))
    a5p_pool = stack.enter_context(tc.tile_pool(name="a5p", bufs=6))
    a5i_pool = stack.enter_context(tc.tile_pool(name="a5i", bufs=6))
    a6_pool = stack.enter_context(tc.tile_pool(name="a6", bufs=6))
    a7_pool = stack.enter_context(tc.tile_pool(name="a7", bufs=8))
    rep_pool = stack.enter_context(tc.tile_pool(name="rep7", bufs=4))
    psA = stack.enter_context(tc.tile_pool(name="psA", bufs=6, space="PSUM"))
    psB = stack.enter_context(tc.tile_pool(name="psB", bufs=2, space="PSUM"))

    wt = {}
    for name, shape, dt in (
        ('w1t', [128, 128], BF16), ('w2t', [128, 9 * 128], BF16),
        ('w3t', [128, 9 * 128], BF16), ('w4t', [128, 9 * 128], BF16),
        ('w5t', [128, 9 * 128], BF16), ('w6t', [128, 9 * 128], BF16),
        ('w7', [128, 54 * 128], BF16), ('w8', [128, 25], F32),
        ('b1', [128, 1], F32), ('b2', [128, 1], F32), ('b3', [128, 1], F32),
        ('b4', [128, 1], F32), ('b5', [128, 1], F32), ('b6', [128, 1], F32),
        ('b7', [128, 1], F32), ('b8', [25, 1], F32),
    ):
        t = persist.tile(shape, dt, tag=name, name=nm(name))
        nc.sync.dma_start(t[:], ins[name])
        wt[name] = t

    PADF = 146 * 17
    pad_tiles = {}
    for l in (2, 3, 4):
        for i in range(4):
            t = persist.tile([128, PADF], BF16, tag=f"act{l}_{i}", name=nm("pad"))
            v = t[:].rearrange("p (h w) -> p h w", h=146, w=17)
            nc.vector.memset(v[:, 0, :], 0.0)
            nc.vector.memset(v[:, 145, :], 0.0)
            nc.vector.memset(v[:, :, 0], 0.0)
            nc.vector.memset(v[:, :, 16], 0.0)
            pad_tiles[(l, i)] = t

    act9 = persist.tile([128, nwin_total], F32, tag="act9", name="act9")
    dummy8 = persist.tile([128, 8 * 69], F32, tag="dummy8", name="dummy8")
    out_sb = persist.tile([25, nwin_total], F32, tag="out_sb", name="out_sb")

    win_order = []

    def evac(eng_idx, dst, src, bias_ap):
        """bias + ReLU + downcast evacuation on alternating engines."""
        if eng_idx == 0:
            nc.scalar.activation(dst, src, RELU, bias=bias_ap)
        else:
            nc.vector.tensor_scalar(out=dst, in0=src, scalar1=bias_ap,
                                    scalar2=0.0, op0=ADD, op1=MAX)

    def conv_pad_layer(l, in_tiles, out_is_pad, bias, w_t):
        n_t = len(in_tiles)
        n_g = max(len(t.windows) for t in in_tiles)
        outs = []
        for g in range(n_g):
            if out_is_pad:
                ot = pad_tiles[(l + 1, g)]
                outs.append(ActTile(ot[:], 32, {}))
            else:
                ot = a5p_pool.tile([128, 144 * 15], BF16, tag="a5p", name=nm("a5p"))
                outs.append(ActTile(ot[:], 32, {}))
        wv = w_t[:].rearrange("p (k c) -> p k c", k=9, c=128)
        for (h0, hc) in CHUNKS_144:
            N = hc * 15
            pss = [psA.tile([128, 512], F32, tag="mm", name=nm("mm")) for _ in range(n_g)]
            for k in range(9):
                dh, dw = k // 3, k % 3
                # dummy 1-col matmul: its auto-LDWEIGHTS is a single FWL
                # 128-col load covering all 16 tiles; anchored to this
                # group's PSUM bank so the scheduler can't hoist it.
                nc.tensor.matmul(pss[0][:, 508:509], wv[:, k, :],
                                 wv[:, k, 0:1], start=False, stop=False,
                                 skip_group_check=True, tile_position=(0, 0))
                for g in range(n_g):
                    for Ti, it in enumerate(in_tiles):
                        if g not in it.windows:
                            continue
                        iv = it.ap.rearrange("p (h w) -> p h w", h=146, w=17)
                        rhs = iv[32 * g:32 * g + 32, h0 + dh:h0 + dh + hc, dw:dw + 15]
                        nc.tensor.matmul(
                            pss[g][32 * Ti:32 * Ti + 32, 0:N],
                            wv[32 * g:32 * g + 32, k, 32 * Ti:32 * Ti + 32],
                            rhs, start=(k == 0), stop=(k == 8), skip_group_check=True,
                            tile_position=(32 * g, 32 * Ti))
            for g in range(n_g):
                np_used = 32 * n_t
                src = pss[g][0:np_used, 0:N].rearrange("p (h w) -> p h w", h=hc, w=15)
                if out_is_pad:
                    ov = outs[g].ap.rearrange("p (h w) -> p h w", h=146, w=17)
                    dst = ov[0:np_used, 1 + h0:1 + h0 + hc, 1:16]
                else:
                    ov = outs[g].ap.rearrange("p (h w) -> p h w", h=144, w=15)
                    dst = ov[0:np_used, h0:h0 + hc, :]
                evac(g % 2, dst, src, bias[0:np_used, :])
        for g in range(n_g):
            for Ti, it in enumerate(in_tiles):
                if g in it.windows:
                    outs[g].windows[Ti] = it.windows[g]
        return outs

    def l1_wave(x9t, t0, ncols, b_idx):
        xv = x9t[:].rearrange("p (c h w) -> p c h w", c=4, h=F, w=WIN)
        outs = [ActTile(pad_tiles[(2, g)][:], 32, {}) for g in range(4)]
        for (h0, hc) in CHUNKS_144:
            N = hc * 15
            pss = [psA.tile([128, 512], F32, tag="mm", name=nm("mm")) for _ in range(4)]
            nc.tensor.matmul(pss[0][:, 508:509], wt['w1t'][:],
                             wt['w1t'][:, 0:1], start=False, stop=False,
                             skip_group_check=True, tile_position=(0, 0))
            for g in range(4):
                for T_ in range(ncols):
                    nc.tensor.matmul(
                        pss[g][32 * T_:32 * T_ + 32, 0:N],
                        wt['w1t'][32 * g:32 * g + 9, 32 * T_:32 * T_ + 32],
                        xv[32 * g:32 * g + 9, T_, h0:h0 + hc, :],
                        start=True, stop=True, skip_group_check=True,
                        tile_position=(32 * g, 32 * T_))
            for g in range(4):
                np_used = 32 * ncols
                src = pss[g][0:np_used, 0:N].rearrange("p (h w) -> p h w", h=hc, w=15)
                ov = outs[g].ap.rearrange("p (h w) -> p h w", h=146, w=17)
                evac(g % 2, ov[0:np_used, 1 + h0:1 + h0 + hc, 1:16], src,
                     wt['b1'][0:np_used, :])
        for g in range(4):
            for T_ in range(ncols):
                outs[g].windows[T_] = b_idx * T + t0 + T_ * 4 + g
        return outs

    def pool1(a5p_tiles):
        outs = []
        for t in a5p_tiles:
            np_used = 32 * (max(t.windows) + 1)
            o = a5i_pool.tile([128, 72 * 15], BF16, tag="a5i", name=nm("a5i"))[:]
            sv = t.ap.rearrange("p (h two w) -> p h two w", h=72, two=2, w=15)
            ov = o.rearrange("p (h w) -> p h w", h=72, w=15)
            nc.vector.tensor_max(ov[0:np_used], sv[0:np_used, :, 0, :],
                                 sv[0:np_used, :, 1, :])
            outs.append(ActTile(o, 32, dict(t.windows)))
        return outs

    def l5_wave(tiles):
        pair = len(tiles) == 2
        a6_tiles = []
        n_g = len(tiles[0].windows)
        for g in range(n_g):
            o = a6_pool.tile([128, 70 * 13], BF16, tag="a6", name=nm("a6"))[:]
            a6_tiles.append(ActTile(o, 64, {}))
        wv = wt['w5t'][:].rearrange("p (k c) -> p k c", k=9, c=128)
        for (h0, hc) in CHUNKS_70:
            N = hc * 13
            pss = [psA.tile([128, 512], F32, tag="mm", name=nm("mm")) for _ in range(n_g)]
            for k in range(9):
                dh, dw = k // 3, k % 3
                nc.tensor.matmul(pss[0][:, 508:509], wv[:, k, :],
                                 wv[:, k, 0:1], start=False, stop=False,
                                 skip_group_check=True, tile_position=(0, 0))
                for g in range(n_g):
                    for c5, it in enumerate(tiles):
                        iv = it.ap.rearrange("p (h w) -> p h w", h=72, w=15)
                        rhs = iv[32 * g:32 * g + 32, h0 + dh:h0 + dh + hc, dw:dw + 13]
                        nc.tensor.matmul(
                            pss[g][64 * c5:64 * c5 + 64, 0:N],
                            wv[32 * g:32 * g + 32, k, 64 * c5:64 * c5 + 64],
                            rhs, start=(k == 0), stop=(k == 8), skip_group_check=True,
                            tile_position=(32 * g, 64 * c5))
            for g in range(n_g):
                np_used = 128 if pair else 64
                src = pss[g][0:np_used, 0:N].rearrange("p (h w) -> p h w", h=hc, w=13)
                ov = a6_tiles[g].ap.rearrange("p (h w) -> p h w", h=70, w=13)
                if pair:
                    dst = ov[:, h0:h0 + hc, :]
                else:
                    dst = ov[0:64, h0:h0 + hc, :]
                nc.scalar.activation(dst, src, RELU, bias=wt['b5'][0:np_used, :])
        for g in range(n_g):
            for c5, it in enumerate(tiles):
                a6_tiles[g].windows[c5] = it.windows[g]
        if not pair:
            full = []
            for j in range(0, n_g, 2):
                t0_, t1_ = a6_tiles[j], a6_tiles[j + 1]
                o = a6_pool.tile([128, 70 * 13], BF16, tag="a6", name=nm("a6"))[:]
                nc.vector.tensor_copy(o[0:64, :], t0_.ap[0:64, :])
                nc.vector.tensor_copy(o[64:128, :], t1_.ap[0:64, :])
                full.append(ActTile(o, 64, {0: t0_.windows[0], 1: t1_.windows[0]}))
            a6_tiles = full
        return a6_tiles

    def l6_wave(tP, tQ):
        a7_tiles = [ActTile(a7_pool.tile([128, 68 * 11], BF16, tag="a7",
                                         name=nm("a7"))[:], 64, {})
                    for _ in range(2)]
        wv = wt['w6t'][:].rearrange("p (k c) -> p k c", k=9, c=128)
        for (h0, hc) in CHUNKS_68:
            N = hc * 11
            pss = [psA.tile([128, 512], F32, tag="mm", name=nm("mm")) for _ in range(2)]
            for k in range(9):
                dh, dw = k // 3, k % 3
                nc.tensor.matmul(pss[0][:, 508:509], wv[:, k, :],
                                 wv[:, k, 0:1], start=False, stop=False,
                                 skip_group_check=True, tile_position=(0, 0))
                for c in range(2):
                    for cc, it in enumerate((tP, tQ)):
                        iv = it.ap.rearrange("p (h w) -> p h w", h=70, w=13)
                        rhs = iv[64 * c:64 * c + 64, h0 + dh:h0 + dh + hc, dw:dw + 11]
                        nc.tensor.matmul(
                            pss[c][64 * cc:64 * cc + 64, 0:N],
                            wv[64 * c:64 * c + 64, k, 64 * cc:64 * cc + 64],
                            rhs, start=(k == 0), stop=(k == 8), skip_group_check=True,
                            tile_position=(64 * c, 64 * cc))
            for c in range(2):
                src = pss[c][:, 0:N].rearrange("p (h w) -> p h w", h=hc, w=11)
                ov = a7_tiles[c].ap.rearrange("p (h w) -> p h w", h=68, w=11)
                nc.scalar.activation(ov[:, h0:h0 + hc, :], src, RELU,
                                     bias=wt['b6'][:])
        for c in range(2):
            a7_tiles[c].windows = {0: tP.windows[c], 1: tQ.windows[c]}
        return a7_tiles

    REP_S = 7
    rep_state = {"tiles": []}

    def rep_flush():
        tiles = rep_state["tiles"]
        rep_state["tiles"] = []
        if not tiles:
            return
        w7v = wt['w7'][:].rearrange("p (i c) -> p i c", i=54, c=128)
        pss = []
        for t in tiles:
            S = len(t["slots"])
            ps7 = psB.tile([128, REP_S * 69], F32, tag="l7", name=nm("ps7"))[:]
            pv = ps7.rearrange("p (s h w) -> p s h w", s=REP_S, h=23, w=3)
            pss.append((t, S, pv))
        for jj in range(6):
            for dw in range(9):
                idx = jj * 9 + dw
                for (t, S, pv) in pss:
                    nc.tensor.matmul(
                        pv[:, 0:S, :, :], w7v[:, idx, :],
                        t["rv"][:, 0:S, 2 * jj:2 * jj + 23, dw:dw + 3],
                        start=(idx == 0), stop=(idx == 53), skip_group_check=True,
                        tile_position=(0, 0))
        dv = dummy8[:].rearrange("p (s n) -> p s n", s=8, n=69)
        for (t, S, pv) in pss:
            for s in range(S):
                col = len(win_order)
                win_order.append(t["slots"][s])
                nc.scalar.activation(
                    dv[:, col % 8, :], pv[:, s, :, :].rearrange("p h w -> p (h w)"),
                    RELU, bias=wt['b7'][:],
                    accum_out=act9[:, col:col + 1])

    def l7_push(tv, c, win):
        """Append one window's pooled conv6 output into the rep tiles."""
        tiles = rep_state["tiles"]
        if not tiles or len(tiles[-1]["slots"]) == REP_S:
            rep = rep_pool.tile([128, REP_S * 33 * 11], BF16, tag="rep7",
                                name=nm("rep7"))[:]
            rv = rep.rearrange("p (s h w) -> p s h w", s=REP_S, h=33, w=11)
            tiles.append({"rv": rv, "slots": []})
        cur = tiles[-1]
        s = len(cur["slots"])
        cur["slots"].append(win)
        for p in range(2):
            nc.vector.tensor_max(
                cur["rv"][64 * p:64 * p + 64, s, :, :],
                tv[64 * c:64 * c + 64, p:p + 33, 0, :],
                tv[64 * c:64 * c + 64, p:p + 33, 1, :])
        if len(tiles) == 2 and len(tiles[-1]["slots"]) == REP_S:
            rep_flush()

    def l7_group(a7_pair):
        for t in a7_pair:
            tv = t.ap.rearrange("p (h two w) -> p h two w", h=34, two=2, w=11)
            for c in range(2):
                l7_push(tv, c, t.windows[c])

    _dbg_row = [0]
    for rep in range(repeat):
        if rep > 0:
            win_order.clear()
        x9_off = 0
        a5i_queue = []
        a6_queue = []

        def drain_a6():
            while len(a6_queue) >= 2:
                tP = a6_queue.pop(0)
                tQ = a6_queue.pop(0)
                a7s = l6_wave(tP, tQ)
                if stop_after == 'l6':
                    for tl in a7s:
                        r = _dbg_row[0] % nwin_total
                        _dbg_row[0] += 1
                        nc.gpsimd.dma_start(
                            y_ap[r:r + 1, 0:25].rearrange("a c -> c a"),
                            tl.ap[0:25, 0:1])
                    continue
                l7_group(a7s)

        def drain_a5(force=False):
            while len(a5i_queue) >= 2 or (force and a5i_queue):
                tiles = [a5i_queue.pop(0)]
                if a5i_queue:
                    tiles.append(a5i_queue.pop(0))
                a6_queue.extend(l5_wave(tiles))
                drain_a6()

        for b_idx in range(n_b):
            for wi, (t0, ncols) in enumerate(plan):
                nelem = 4 * 9 * ncols * F * WIN
                x9t = x9pool.tile([128, 4 * F * WIN], BF16, tag="x9", name=nm("x9"))
                per_g = nelem // 4
                for g in range(4):
                    src = ins['x9'][x9_off + g * per_g: x9_off + (g + 1) * per_g]
                    dst = x9t[:].rearrange("p (c h w) -> p c h w", c=4, h=F, w=WIN)
                    nc.sync.dma_start(
                        dst[32 * g:32 * g + 9, 0:ncols, :, :],
                        src.rearrange("(k c h w) -> k c h w", k=9, c=ncols, h=F, w=WIN))
                x9_off += nelem
                tiles = l1_wave(x9t, t0, ncols, b_idx)
                tiles = conv_pad_layer(2, tiles, True, wt['b2'], wt['w2t'])
                tiles = conv_pad_layer(3, tiles, True, wt['b3'], wt['w3t'])
                tiles = conv_pad_layer(4, tiles, False, wt['b4'], wt['w4t'])
                a5i_queue.extend(pool1(tiles))
                if stop_after == 'l4':
                    while a5i_queue:
                        tl = a5i_queue.pop(0)
                        r = _dbg_row[0] % nwin_total
                        _dbg_row[0] += 1
                        nc.gpsimd.dma_start(
                            y_ap[r:r + 1, 0:25].rearrange("a c -> c a"),
                            tl.ap[0:25, 0:1])
                    continue
                drain_a5()
            drain_a5(force=True)
            drain_a6()
        if stop_after is not None:
            a5i_queue.clear()
            a6_queue.clear()
            continue
        rep_flush()
        assert not a5i_queue and not a6_queue
        assert len(win_order) == nwin_total

        ps8 = psB.tile([25, nwin_total], F32, tag="l7", name=nm("ps8"))
        nc.tensor.matmul(ps8[:, :], wt['w8'][:, 0:25], act9[:, :],
                         start=True, stop=True)
        nc.scalar.activation(out_sb[:, :], ps8[:, :], IDENT, bias=wt['b8'][:])
        nc.sync.dma_start(y_ap.rearrange("t c -> c t"), out_sb[:, :])

    stack.close()
    return win_order


# --------------------------------------------------------------- entry point

_CACHE = {}


def build_program(in_map, n_b=2, T=T_PER_B, repeat=1, stop_after=None):
    """Build + bacc-compile the SPMD program. Returns (nc, win_order)."""
    nc = bacc.Bacc("TRN2", target_bir_lowering=False, debug=False,
                   num_devices=N_CORES)
    dram = {}
    for name, arr in in_map.items():
        dram[name] = nc.dram_tensor(name, list(arr.shape),
                                    mybir.dt.from_np(arr.dtype),
                                    kind="ExternalInput")
    nwin = n_b * T
    y = nc.dram_tensor("y", [nwin, 25], mybir.dt.float32, kind="ExternalOutput")
    with tile.TileContext(nc) as tc:
        win_order = emit(tc, {k: v.ap() for k, v in dram.items()}, y.ap(),
                         n_b=n_b, T=T, repeat=repeat, stop_after=stop_after)
    if DEDUPE_LDW:
        dedupe_ldweights(nc)
    nc.compile()
    return nc, win_order


def kernel(**inputs):
    x = np.asarray(inputs['x'])
    B, T, _ = x.shape
    in_maps, _plan = host_prepare(inputs, n_cores=N_CORES)
    key = (B, T)
    if key not in _CACHE:
        _CACHE[key] = build_program(in_maps[0], n_b=B // N_CORES, T=T)
    nc, win_order = _CACHE[key]
    res = run_bass_kernel_spmd(nc, in_maps, list(range(N_CORES)))
    order = np.asarray(win_order)
    b_per_core = B // N_CORES
    out = np.zeros((B, T, 25), np.float32)
    for c in range(N_CORES):
        yc = np.zeros((b_per_core * T, 25), np.float32)
        yc[order] = res.results[c]['y']
        out[c * b_per_core:(c + 1) * b_per_core] = yc.reshape(b_per_core, T, 25)
    return out
